# revision 5
# baseline (speedup 1.0000x reference)
"""Multi-head causal self-attention (B=64, T=256, C=384, H=6) on 8 NeuronCores.

Data-parallel over batch: each core processes 8 batches (2048 tokens).
All on-device tensors are laid out so no device-side transposes are needed:
  - xT, Q.T, K.T feature-major [C, tokens]
  - V token-major [tokens, C]
  - scores computed transposed (S.T[tk, tq]) so exp(S.T) feeds P.T@V directly
  - attention output lands feature-major (catT) for the output projection
"""

import sys

import numpy as np

for _p in ("/opt/trn_rl_repo", "/root/.axon_site/_ro/trn_rl_repo"):
    if _p not in sys.path:
        sys.path.insert(0, _p)

import concourse.bass as bass
import concourse.tile as tile
from concourse import bacc, mybir
from concourse.bass_utils import run_bass_kernel_spmd

B, T, C, H, D = 64, 256, 384, 6, 64
NCORES = 8
BB = B // NCORES  # batches per core = 8
TOK = BB * T      # tokens per core = 2048
SCALE = float(C) ** -0.5
F32 = mybir.dt.float32

NT4 = TOK // 512  # 4 column-chunks of 512 tokens
NKC = C // 128    # 3 chunks of 128 over feature dim


def build_module():
    nc = bacc.Bacc("TRN2", target_bir_lowering=False, debug=False)

    xT = nc.dram_tensor("xT", [C, TOK], F32, kind="ExternalInput").ap()
    wqT = nc.dram_tensor("wqT", [C, C], F32, kind="ExternalInput").ap()
    wkT = nc.dram_tensor("wkT", [C, C], F32, kind="ExternalInput").ap()
    wvT = nc.dram_tensor("wvT", [C, C], F32, kind="ExternalInput").ap()
    woT = nc.dram_tensor("woT", [C, C], F32, kind="ExternalInput").ap()
    wobc = nc.dram_tensor("wobc", [C, 1], F32, kind="ExternalInput").ap()
    ez = nc.dram_tensor("ez", [128, H * H], F32, kind="ExternalInput").ap()
    yT = nc.dram_tensor("yT", [C, TOK], F32, kind="ExternalOutput").ap()

    with tile.TileContext(nc) as tc:
        import contextlib

        ctx = contextlib.ExitStack()
        with ctx:
            consts = ctx.enter_context(tc.tile_pool(name="consts", bufs=1))

            # ---- persistent SBUF tiles ----
            def ptile(name, shape):
                return consts.tile(shape, F32, tag=name, name=name)

            wq_sb = [ptile(f"wq{k}", [128, C]) for k in range(NKC)]
            wk_sb = [ptile(f"wk{k}", [128, C]) for k in range(NKC)]
            wv_sb = [ptile(f"wv{k}", [128, C]) for k in range(NKC)]
            wo_sb = [ptile(f"wo{k}", [128, C]) for k in range(NKC)]
            wob_sb = [ptile(f"wob{k}", [128, 1]) for k in range(NKC)]
            ez_sb = ptile("ez", [128, H * H])
            xt_sb = [[ptile(f"xt{k}_{t}", [128, 512]) for t in range(NT4)] for k in range(NKC)]
            qt_sb = [[ptile(f"qt{k}_{t}", [128, 512]) for t in range(NT4)] for k in range(NKC)]
            kt_sb = [[ptile(f"kt{k}_{t}", [128, 512]) for t in range(NT4)] for k in range(NKC)]
            cat_sb = [[ptile(f"cat{k}_{t}", [128, 512]) for t in range(NT4)] for k in range(NKC)]
            v_sb = [ptile(f"v{t}", [128, C]) for t in range(2 * BB)]  # 16 token-blocks of 128

            # ---- input DMAs ----
            for k in range(NKC):
                nc.sync.dma_start(out=wq_sb[k], in_=wqT[128 * k:128 * (k + 1), :])
                nc.sync.dma_start(out=wk_sb[k], in_=wkT[128 * k:128 * (k + 1), :])
                nc.sync.dma_start(out=wv_sb[k], in_=wvT[128 * k:128 * (k + 1), :])
                nc.sync.dma_start(out=wo_sb[k], in_=woT[128 * k:128 * (k + 1), :])
                nc.sync.dma_start(out=wob_sb[k], in_=wobc[128 * k:128 * (k + 1), :])
            nc.sync.dma_start(out=ez_sb, in_=ez)
            for k in range(NKC):
                for t in range(NT4):
                    nc.sync.dma_start(
                        out=xt_sb[k][t],
                        in_=xT[128 * k:128 * (k + 1), 512 * t:512 * (t + 1)],
                    )

            # ---- PSUM pools ----
            pa = ctx.enter_context(tc.tile_pool(name="pa", bufs=3, space="PSUM"))
            ps = ctx.enter_context(tc.tile_pool(name="ps", bufs=2, space="PSUM"))
            po = ctx.enter_context(tc.tile_pool(name="po", bufs=3, space="PSUM"))

            # working SBUF pools
            pt_pool = ctx.enter_context(tc.tile_pool(name="ptp", bufs=4))
            rp_pool = ctx.enter_context(tc.tile_pool(name="rpp", bufs=3))
            bc_pool = ctx.enter_context(tc.tile_pool(name="bcp", bufs=6))
            y_pool = ctx.enter_context(tc.tile_pool(name="yp", bufs=3))

            # ---- phase 1a: Q.T / K.T = W @ x.T, feature-major [C, tok] ----
            for which, w_sb, out_sb in (("q", wq_sb, qt_sb), ("k", wk_sb, kt_sb)):
                for co in range(NKC):
                    for t in range(NT4):
                        pqk = pa.tile([128, 512], F32, tag="pa", name=f"p{which}{co}_{t}")
                        for kc in range(NKC):
                            nc.tensor.matmul(
                                pqk,
                                wq_sb[kc][:, 128 * co:128 * (co + 1)] if which == "q"
                                else wk_sb[kc][:, 128 * co:128 * (co + 1)],
                                xt_sb[kc][t],
                                start=(kc == 0),
                                stop=(kc == NKC - 1),
                            )
                        if which == "q":
                            nc.scalar.copy(out_sb[co][t], pqk)
                        else:
                            nc.vector.tensor_copy(out_sb[co][t], pqk)

            # ---- phase 1b: V token-major [tok, C] ----
            for tb in range(2 * BB):
                pv = pa.tile([128, C], F32, tag="pa", name=f"pv{tb}")
                t4, off = tb // 4, (tb % 4) * 128
                for kc in range(NKC):
                    nc.tensor.matmul(
                        pv,
                        xt_sb[kc][t4][:, off:off + 128],
                        wv_sb[kc],
                        start=(kc == 0),
                        stop=(kc == NKC - 1),
                    )
                nc.scalar.copy(v_sb[tb], pv)

            # ---- phase 2: attention per (batch, head) ----
            for b in range(BB):
                t4b, qc = b // 2, (b % 2) * 256  # which 512-tile / col offset for this batch
                pzz = pa.tile([H, 256], F32, tag="pa", name=f"pzz{b}")
                po_tiles = []
                for hp in range(H // 2):  # head pairs
                    p_o = po.tile([64, 512], F32, tag="po", name=f"po{b}_{hp}")
                    po_tiles.append(p_o)
                    for hh in range(2):
                        h = 2 * hp + hh
                        r0 = 64 * hh
                        c0 = 256 * hh
                        qt = qt_sb[hp][t4b]
                        kt = kt_sb[hp][t4b]
                        # scores transposed: S.T[tk, tq] = K.T(d,tk).T @ Q.T(d,tq)
                        p_s = ps.tile([128, 384], F32, tag="ps", name=f"s{b}_{h}")
                        nc.tensor.matmul(
                            p_s[:, 0:256],
                            kt[r0:r0 + 64, qc:qc + 128],
                            qt[r0:r0 + 64, qc:qc + 256],
                            start=True, stop=True,
                        )
                        nc.tensor.matmul(
                            p_s[:, 256:384],
                            kt[r0:r0 + 64, qc + 128:qc + 256],
                            qt[r0:r0 + 64, qc + 128:qc + 256],
                            start=True, stop=True,
                        )
                        # P.T = exp(S.T / sqrt(C)); cols 0:256 = tk-blk0 x tq 0:256,
                        # cols 256:384 = tk-blk1 x tq 128:256
                        pt = pt_pool.tile([128, 384], F32, tag="pt", name=f"pt{b}_{h}")
                        nc.scalar.activation(pt, p_s, mybir.ActivationFunctionType.Exp, scale=SCALE)
                        # causal mask on the two diagonal blocks (cols 0:128 and 256:384):
                        # keep where tq_within_block >= tk, else 0
                        sel = pt.rearrange("p (a i) -> p a i", i=128)[:, 0::2, :]
                        nc.gpsimd.affine_select(
                            out=sel, in_=sel,
                            pattern=[[0, 2], [1, 128]],
                            compare_op=mybir.AluOpType.is_ge,
                            fill=0.0, base=0, channel_multiplier=-1,
                        )
                        # O.T[d, tq] (+= over tk blocks)
                        nc.tensor.matmul(
                            p_o[0:64, c0:c0 + 256],
                            v_sb[2 * b][:, 64 * h:64 * (h + 1)],
                            pt[:, 0:256],
                            start=True, stop=False,
                        )
                        nc.tensor.matmul(
                            p_o[0:64, c0 + 128:c0 + 256],
                            v_sb[2 * b + 1][:, 64 * h:64 * (h + 1)],
                            pt[:, 256:384],
                            start=False, stop=True,
                        )
                        # Z-gather: row h of pzz accumulates softmax denominators
                        nc.tensor.matmul(
                            pzz[0:H, 0:256],
                            ez_sb[:, H * h:H * (h + 1)],
                            pt[:, 0:256],
                            start=(h == 0), stop=False,
                            skip_group_check=True,
                        )
                        nc.tensor.matmul(
                            pzz[0:H, 128:256],
                            ez_sb[:, H * h:H * (h + 1)],
                            pt[:, 256:384],
                            start=False, stop=(h == H - 1),
                            skip_group_check=True,
                        )
                # denominators -> reciprocals (one op for all 6 heads of this batch)
                rp = rp_pool.tile([H, 256], F32, tag="rp", name=f"rp{b}")
                nc.vector.reciprocal_approx_fast(rp, pzz)
                # normalize each head's O.T into catT
                for h in range(H):
                    hp, hh = h // 2, h % 2
                    c0 = 256 * hh
                    bc = bc_pool.tile([64, 256], F32, tag="bc", name=f"bc{b}_{h}")
                    src = rp[h:h + 1, :]
                    # replicate the [1,256] reciprocal row 64x: keep the (count-1)
                    # partition dim, add a step-0 free dim of count 64
                    bc_src = bass.AP(
                        tensor=src.tensor, offset=src.offset,
                        ap=[list(src.ap[0]), [0, 64]] + [list(p) for p in src.ap[1:]],
                    )
                    nc.sync.dma_start(out=bc, in_=bc_src)
                    nc.vector.tensor_mul(
                        cat_sb[hp][t4b][64 * hh:64 * (hh + 1), qc:qc + 256],
                        po_tiles[hp][0:64, c0:c0 + 256],
                        bc,
                    )

            # ---- phase 3: y.T = Wo @ catT + bo ----
            for co in range(NKC):
                for t in range(NT4):
                    pyk = pa.tile([128, 512], F32, tag="pa", name=f"py{co}_{t}")
                    for kc in range(NKC):
                        nc.tensor.matmul(
                            pyk,
                            wo_sb[kc][:, 128 * co:128 * (co + 1)],
                            cat_sb[kc][t],
                            start=(kc == 0),
                            stop=(kc == NKC - 1),
                        )
                    yt = y_pool.tile([128, 512], F32, tag="yt", name=f"yt{co}_{t}")
                    nc.vector.tensor_scalar_add(yt, pyk, wob_sb[co][:, 0:1])
                    nc.sync.dma_start(
                        out=yT[128 * co:128 * (co + 1), 512 * t:512 * (t + 1)],
                        in_=yt,
                    )

    nc.compile()
    return nc


def make_in_maps(x, Wk, Wq, Wv, Wo, bo):
    x = np.asarray(x, np.float32)
    wqT = np.ascontiguousarray(np.asarray(Wq, np.float32).T)
    wkT = np.ascontiguousarray(np.asarray(Wk, np.float32).T)
    wvT = np.ascontiguousarray(np.asarray(Wv, np.float32).T)
    woT = np.ascontiguousarray(np.asarray(Wo, np.float32).T)
    wobc = np.ascontiguousarray(np.asarray(bo, np.float32).reshape(C, 1))
    ez = np.zeros((128, H * H), np.float32)
    for h in range(H):
        ez[:, H * h + h] = 1.0
    in_maps = []
    for i in range(NCORES):
        xi = x[BB * i:BB * (i + 1)].reshape(TOK, C)
        in_maps.append({
            "xT": np.ascontiguousarray(xi.T),
            "wqT": wqT, "wkT": wkT, "wvT": wvT, "woT": woT,
            "wobc": wobc, "ez": ez,
        })
    return in_maps


_NC_CACHE = None


def kernel(x, Wk, Wq, Wv, Wo, bo):
    global _NC_CACHE
    if _NC_CACHE is None:
        _NC_CACHE = build_module()
    nc = _NC_CACHE
    in_maps = make_in_maps(x, Wk, Wq, Wv, Wo, bo)
    res = run_bass_kernel_spmd(nc, in_maps, core_ids=list(range(NCORES)))
    outs = []
    for i in range(NCORES):
        yt = np.asarray(res.results[i]["yT"])
        outs.append(yt.T.reshape(BB, T, C))
    return np.concatenate(outs, axis=0).astype(np.float32)


# revision 7
# speedup vs baseline: 2.2323x; 2.2323x over previous
"""Multi-head causal self-attention (B=64, T=256, C=384, H=6) on 8 NeuronCores.

Data-parallel over batch: each core processes 8 batches (2048 tokens).
All on-device tensors are laid out so no device-side transposes are needed:
  - xT, Q.T, K.T feature-major [C, tokens]
  - V token-major [tokens, C]
  - scores computed transposed (S.T[tk, tq]) so exp(S.T) feeds P.T@V directly
  - attention output lands feature-major (catT) for the output projection
Matmul operands are bf16 (fp32 matmul is two-pass on trn2); accumulation,
softmax denominators, normalization and the final output stay fp32.
"""

import sys

import ml_dtypes
import numpy as np

for _p in ("/opt/trn_rl_repo", "/root/.axon_site/_ro/trn_rl_repo"):
    if _p not in sys.path:
        sys.path.insert(0, _p)

import concourse.bass as bass
import concourse.tile as tile
from concourse import bacc, mybir
from concourse.bass_utils import run_bass_kernel_spmd

B, T, C, H, D = 64, 256, 384, 6, 64
NCORES = 8
BB = B // NCORES  # batches per core = 8
TOK = BB * T      # tokens per core = 2048
SCALE = float(C) ** -0.5
F32 = mybir.dt.float32
BF16 = mybir.dt.bfloat16
NPBF = ml_dtypes.bfloat16

NT4 = TOK // 512  # 4 column-chunks of 512 tokens
NKC = C // 128    # 3 chunks of 128 over feature dim


def build_module():
    nc = bacc.Bacc("TRN2", target_bir_lowering=False, debug=False)

    xT = nc.dram_tensor("xT", [C, TOK], BF16, kind="ExternalInput").ap()
    wqT = nc.dram_tensor("wqT", [C, C], BF16, kind="ExternalInput").ap()
    wkT = nc.dram_tensor("wkT", [C, C], BF16, kind="ExternalInput").ap()
    wvT = nc.dram_tensor("wvT", [C, C], BF16, kind="ExternalInput").ap()
    woT = nc.dram_tensor("woT", [C, C], BF16, kind="ExternalInput").ap()
    wobc = nc.dram_tensor("wobc", [C, 1], F32, kind="ExternalInput").ap()
    ez = nc.dram_tensor("ez", [128, H * H], BF16, kind="ExternalInput").ap()
    tril = nc.dram_tensor("tril", [128, 256], BF16, kind="ExternalInput").ap()
    yT = nc.dram_tensor("yT", [C, TOK], F32, kind="ExternalOutput").ap()

    with tile.TileContext(nc) as tc:
        import contextlib

        ctx = contextlib.ExitStack()
        with ctx:
            consts = ctx.enter_context(tc.tile_pool(name="consts", bufs=1))

            # ---- persistent SBUF tiles ----
            def ptile(name, shape, dt=BF16):
                return consts.tile(shape, dt, tag=name, name=name)

            wq_sb = [ptile(f"wq{k}", [128, C]) for k in range(NKC)]
            wk_sb = [ptile(f"wk{k}", [128, C]) for k in range(NKC)]
            wv_sb = [ptile(f"wv{k}", [128, C]) for k in range(NKC)]
            wo_sb = [ptile(f"wo{k}", [128, C]) for k in range(NKC)]
            wob_sb = [ptile(f"wob{k}", [128, 1], F32) for k in range(NKC)]
            ez_sb = ptile("ez", [128, H * H])
            tril_sb = ptile("tril", [128, 256])
            xt_sb = [[ptile(f"xt{k}_{t}", [128, 512]) for t in range(NT4)] for k in range(NKC)]
            qt_sb = [[ptile(f"qt{k}_{t}", [128, 512]) for t in range(NT4)] for k in range(NKC)]
            kt_sb = [[ptile(f"kt{k}_{t}", [128, 512]) for t in range(NT4)] for k in range(NKC)]
            cat_sb = [[ptile(f"cat{k}_{t}", [128, 512]) for t in range(NT4)] for k in range(NKC)]
            v_sb = [ptile(f"v{t}", [128, C]) for t in range(2 * BB)]  # 16 token-blocks of 128

            # ---- input DMAs (weights on gpsimd queue, x on sync queue) ----
            for k in range(NKC):
                nc.gpsimd.dma_start(out=wq_sb[k], in_=wqT[128 * k:128 * (k + 1), :])
                nc.gpsimd.dma_start(out=wk_sb[k], in_=wkT[128 * k:128 * (k + 1), :])
                nc.gpsimd.dma_start(out=wv_sb[k], in_=wvT[128 * k:128 * (k + 1), :])
                nc.gpsimd.dma_start(out=wo_sb[k], in_=woT[128 * k:128 * (k + 1), :])
                nc.gpsimd.dma_start(out=wob_sb[k], in_=wobc[128 * k:128 * (k + 1), :])
            nc.gpsimd.dma_start(out=ez_sb, in_=ez)
            nc.gpsimd.dma_start(out=tril_sb, in_=tril)
            for t in range(NT4):
                for k in range(NKC):
                    nc.sync.dma_start(
                        out=xt_sb[k][t],
                        in_=xT[128 * k:128 * (k + 1), 512 * t:512 * (t + 1)],
                    )

            # ---- PSUM pools ----
            pa = ctx.enter_context(tc.tile_pool(name="pa", bufs=3, space="PSUM"))
            ps = ctx.enter_context(tc.tile_pool(name="ps", bufs=2, space="PSUM"))
            po = ctx.enter_context(tc.tile_pool(name="po", bufs=3, space="PSUM"))

            # working SBUF pools
            pt_pool = ctx.enter_context(tc.tile_pool(name="ptp", bufs=6))
            rp_pool = ctx.enter_context(tc.tile_pool(name="rpp", bufs=3))
            bc_pool = ctx.enter_context(tc.tile_pool(name="bcp", bufs=8))
            y_pool = ctx.enter_context(tc.tile_pool(name="yp", bufs=3))

            # ---- phase 1a: Q.T / K.T = W @ x.T, feature-major [C, tok] ----
            for which, w_sb, out_sb in (("q", wq_sb, qt_sb), ("k", wk_sb, kt_sb)):
                for t in range(NT4):
                    for co in range(NKC):
                        pqk = pa.tile([128, 512], F32, tag="pa", name=f"p{which}{co}_{t}")
                        for kc in range(NKC):
                            nc.tensor.matmul(
                                pqk,
                                w_sb[kc][:, 128 * co:128 * (co + 1)],
                                xt_sb[kc][t],
                                start=(kc == 0),
                                stop=(kc == NKC - 1),
                            )
                        if which == "q":
                            nc.scalar.copy(out_sb[co][t], pqk)
                        else:
                            nc.vector.tensor_copy(out_sb[co][t], pqk)

            # ---- phase 1b: V token-major [tok, C] ----
            for tb in range(2 * BB):
                pv = pa.tile([128, C], F32, tag="pa", name=f"pv{tb}")
                t4, off = tb // 4, (tb % 4) * 128
                for kc in range(NKC):
                    nc.tensor.matmul(
                        pv,
                        xt_sb[kc][t4][:, off:off + 128],
                        wv_sb[kc],
                        start=(kc == 0),
                        stop=(kc == NKC - 1),
                    )
                nc.scalar.copy(v_sb[tb], pv)

            # ---- phase 2: attention per (batch, head) ----
            for b in range(BB):
                t4b, qc = b // 2, (b % 2) * 256  # 512-tile index / col offset for this batch
                pzz = pa.tile([H, 256], F32, tag="pa", name=f"pzz{b}")
                po_tiles = []
                for hp in range(H // 2):  # head pairs
                    p_o = po.tile([64, 512], F32, tag="po", name=f"po{b}_{hp}")
                    po_tiles.append(p_o)
                    for hh in range(2):
                        h = 2 * hp + hh
                        r0 = 64 * hh
                        c0 = 256 * hh
                        qt = qt_sb[hp][t4b]
                        kt = kt_sb[hp][t4b]
                        # scores transposed: S.T[tk, tq] = K.T(d,tk).T @ Q.T(d,tq)
                        p_s = ps.tile([128, 384], F32, tag="ps", name=f"s{b}_{h}")
                        nc.tensor.matmul(
                            p_s[:, 0:256],
                            kt[r0:r0 + 64, qc:qc + 128],
                            qt[r0:r0 + 64, qc:qc + 256],
                            start=True, stop=True,
                        )
                        nc.tensor.matmul(
                            p_s[:, 256:384],
                            kt[r0:r0 + 64, qc + 128:qc + 256],
                            qt[r0:r0 + 64, qc + 128:qc + 256],
                            start=True, stop=True,
                        )
                        # P.T = exp(S.T / sqrt(C)); cols 0:256 = tk-blk0 x tq 0:256,
                        # cols 256:384 = tk-blk1 x tq 128:256
                        pt = pt_pool.tile([128, 384], BF16, tag="pt", name=f"pt{b}_{h}")
                        nc.scalar.activation(pt, p_s, mybir.ActivationFunctionType.Exp, scale=SCALE)
                        # causal mask on the two diagonal blocks (cols 0:128, 256:384)
                        sel = pt.rearrange("p (a i) -> p a i", i=128)[:, 0::2, :]
                        msk = tril_sb.rearrange("p (a i) -> p a i", i=128)
                        nc.vector.tensor_mul(sel, sel, msk)
                        # O.T[d, tq] (+= over tk blocks)
                        nc.tensor.matmul(
                            p_o[0:64, c0:c0 + 256],
                            v_sb[2 * b][:, 64 * h:64 * (h + 1)],
                            pt[:, 0:256],
                            start=True, stop=False,
                        )
                        nc.tensor.matmul(
                            p_o[0:64, c0 + 128:c0 + 256],
                            v_sb[2 * b + 1][:, 64 * h:64 * (h + 1)],
                            pt[:, 256:384],
                            start=False, stop=True,
                        )
                        # Z-gather: row h of pzz accumulates softmax denominators
                        nc.tensor.matmul(
                            pzz[0:H, 0:256],
                            ez_sb[:, H * h:H * (h + 1)],
                            pt[:, 0:256],
                            start=(h == 0), stop=False,
                            skip_group_check=True,
                        )
                        nc.tensor.matmul(
                            pzz[0:H, 128:256],
                            ez_sb[:, H * h:H * (h + 1)],
                            pt[:, 256:384],
                            start=False, stop=(h == H - 1),
                            skip_group_check=True,
                        )
                # denominators -> reciprocals (one op for all 6 heads of this batch)
                rp = rp_pool.tile([H, 256], F32, tag="rp", name=f"rp{b}")
                nc.vector.reciprocal_approx_fast(rp, pzz)
                # normalize each head's O.T into catT
                for h in range(H):
                    hp, hh = h // 2, h % 2
                    c0 = 256 * hh
                    bc = bc_pool.tile([64, 256], F32, tag="bc", name=f"bc{b}_{h}")
                    src = rp[h:h + 1, :]
                    # replicate the [1,256] reciprocal row 64x via step-0 free dim
                    bc_src = bass.AP(
                        tensor=src.tensor, offset=src.offset,
                        ap=[list(src.ap[0]), [0, 64]] + [list(p) for p in src.ap[1:]],
                    )
                    nc.gpsimd.dma_start(out=bc, in_=bc_src)
                    nc.vector.tensor_mul(
                        cat_sb[hp][t4b][64 * hh:64 * (hh + 1), qc:qc + 256],
                        po_tiles[hp][0:64, c0:c0 + 256],
                        bc,
                    )

            # ---- phase 3: y.T = Wo @ catT + bo ----
            for t in range(NT4):
                for co in range(NKC):
                    pyk = pa.tile([128, 512], F32, tag="pa", name=f"py{co}_{t}")
                    for kc in range(NKC):
                        nc.tensor.matmul(
                            pyk,
                            wo_sb[kc][:, 128 * co:128 * (co + 1)],
                            cat_sb[kc][t],
                            start=(kc == 0),
                            stop=(kc == NKC - 1),
                        )
                    yt = y_pool.tile([128, 512], F32, tag="yt", name=f"yt{co}_{t}")
                    nc.vector.tensor_scalar_add(yt, pyk, wob_sb[co][:, 0:1])
                    nc.sync.dma_start(
                        out=yT[128 * co:128 * (co + 1), 512 * t:512 * (t + 1)],
                        in_=yt,
                    )

    nc.compile()
    return nc


def make_in_maps(x, Wk, Wq, Wv, Wo, bo):
    x = np.asarray(x, np.float32)
    wqT = np.ascontiguousarray(np.asarray(Wq, np.float32).T).astype(NPBF)
    wkT = np.ascontiguousarray(np.asarray(Wk, np.float32).T).astype(NPBF)
    wvT = np.ascontiguousarray(np.asarray(Wv, np.float32).T).astype(NPBF)
    woT = np.ascontiguousarray(np.asarray(Wo, np.float32).T).astype(NPBF)
    wobc = np.ascontiguousarray(np.asarray(bo, np.float32).reshape(C, 1))
    ez = np.zeros((128, H * H), NPBF)
    for h in range(H):
        ez[:, H * h + h] = 1.0
    # mask keeps tq >= tk in [tk, tq] layout -> upper triangular incl diagonal
    tl = np.triu(np.ones((128, 128), np.float32))
    tril = np.concatenate([tl, tl], axis=1).astype(NPBF)
    in_maps = []
    for i in range(NCORES):
        xi = x[BB * i:BB * (i + 1)].reshape(TOK, C)
        in_maps.append({
            "xT": np.ascontiguousarray(xi.T).astype(NPBF),
            "wqT": wqT, "wkT": wkT, "wvT": wvT, "woT": woT,
            "wobc": wobc, "ez": ez, "tril": tril,
        })
    return in_maps


_NC_CACHE = None


def kernel(x, Wk, Wq, Wv, Wo, bo):
    global _NC_CACHE
    if _NC_CACHE is None:
        _NC_CACHE = build_module()
    nc = _NC_CACHE
    in_maps = make_in_maps(x, Wk, Wq, Wv, Wo, bo)
    res = run_bass_kernel_spmd(nc, in_maps, core_ids=list(range(NCORES)))
    outs = []
    for i in range(NCORES):
        yt = np.asarray(res.results[i]["yT"])
        outs.append(yt.T.reshape(BB, T, C))
    return np.concatenate(outs, axis=0).astype(np.float32)


# revision 12
# speedup vs baseline: 3.6575x; 1.6385x over previous
"""Multi-head causal self-attention (B=64, T=256, C=384, H=6) on 8 NeuronCores.

Data-parallel over batch: each core processes 8 batches (2048 tokens).
All on-device tensors are laid out so no device-side transposes are needed:
  - xT, Q.T, K.T feature-major [C, tokens]
  - V token-major [tokens, C]
  - scores computed transposed (S.T[tk, tq]) so exp(S.T) feeds P.T@V directly
  - attention output lands feature-major (catT) for the output projection
Matmul operands are bf16 (fp32 matmul is two-pass on trn2); accumulation,
softmax denominators, normalization and the final output stay fp32.
"""

import sys

import ml_dtypes
import numpy as np

for _p in ("/opt/trn_rl_repo", "/root/.axon_site/_ro/trn_rl_repo"):
    if _p not in sys.path:
        sys.path.insert(0, _p)

import concourse.bass as bass
import concourse.tile as tile
from concourse import bacc, mybir
from concourse.bass_utils import run_bass_kernel_spmd

B, T, C, H, D = 64, 256, 384, 6, 64
NCORES = 8
BB = B // NCORES  # batches per core = 8
TOK = BB * T      # tokens per core = 2048
SCALE = float(C) ** -0.5
F32 = mybir.dt.float32
BF16 = mybir.dt.bfloat16
NPBF = ml_dtypes.bfloat16

NT4 = TOK // 512  # 4 column-chunks of 512 tokens
NKC = C // 128    # 3 chunks of 128 over feature dim


def build_module():
    nc = bacc.Bacc("TRN2", target_bir_lowering=False, debug=False)

    xT = nc.dram_tensor("xT", [C, TOK], BF16, kind="ExternalInput").ap()
    wqT = nc.dram_tensor("wqT", [C, C], BF16, kind="ExternalInput").ap()
    wkT = nc.dram_tensor("wkT", [C, C], BF16, kind="ExternalInput").ap()
    wvT = nc.dram_tensor("wvT", [C, C], BF16, kind="ExternalInput").ap()
    woT = nc.dram_tensor("woT", [C, C], BF16, kind="ExternalInput").ap()
    wobc = nc.dram_tensor("wobc", [C, 1], F32, kind="ExternalInput").ap()
    ez = nc.dram_tensor("ez", [128, H * H], BF16, kind="ExternalInput").ap()
    tril = nc.dram_tensor("tril", [128, 256], BF16, kind="ExternalInput").ap()
    yT = nc.dram_tensor("yT", [C, TOK], F32, kind="ExternalOutput").ap()
    # DRAM scratch for the per-batch reciprocal rows (enables broadcast DMA)
    zdram = nc.dram_tensor("zdram", [BB, H * 256], F32).ap()

    with tile.TileContext(nc) as tc:
        import contextlib

        ctx = contextlib.ExitStack()
        with ctx:
            consts = ctx.enter_context(tc.tile_pool(name="consts", bufs=1))

            # ---- persistent SBUF tiles ----
            def ptile(name, shape, dt=BF16):
                return consts.tile(shape, dt, tag=name, name=name)

            wq_sb = [ptile(f"wq{k}", [128, C]) for k in range(NKC)]
            wk_sb = [ptile(f"wk{k}", [128, C]) for k in range(NKC)]
            wv_sb = [ptile(f"wv{k}", [128, C]) for k in range(NKC)]
            wo_sb = [ptile(f"wo{k}", [128, C]) for k in range(NKC)]
            wob_sb = [ptile(f"wob{k}", [128, 1], F32) for k in range(NKC)]
            ez_sb = ptile("ez", [128, H * H])
            tril_sb = ptile("tril", [128, 256])
            xt_sb = [[ptile(f"xt{k}_{t}", [128, 512]) for t in range(NT4)] for k in range(NKC)]
            qt_sb = [[ptile(f"qt{k}_{t}", [128, 512]) for t in range(NT4)] for k in range(NKC)]
            kt_sb = [[ptile(f"kt{k}_{t}", [128, 512]) for t in range(NT4)] for k in range(NKC)]
            cat_sb = [[ptile(f"cat{k}_{t}", [128, 512]) for t in range(NT4)] for k in range(NKC)]
            v_sb = [ptile(f"v{t}", [128, C]) for t in range(2 * BB)]  # 16 token-blocks of 128

            # ---- input DMAs (weights on gpsimd queue, x on sync queue) ----
            for k in range(NKC):
                nc.gpsimd.dma_start(out=wq_sb[k], in_=wqT[128 * k:128 * (k + 1), :])
                nc.gpsimd.dma_start(out=wk_sb[k], in_=wkT[128 * k:128 * (k + 1), :])
                nc.gpsimd.dma_start(out=wv_sb[k], in_=wvT[128 * k:128 * (k + 1), :])
                nc.gpsimd.dma_start(out=wo_sb[k], in_=woT[128 * k:128 * (k + 1), :])
                nc.gpsimd.dma_start(out=wob_sb[k], in_=wobc[128 * k:128 * (k + 1), :])
            nc.gpsimd.dma_start(out=ez_sb, in_=ez)
            nc.gpsimd.dma_start(out=tril_sb, in_=tril)
            for t in range(NT4):
                for k in range(NKC):
                    nc.sync.dma_start(
                        out=xt_sb[k][t],
                        in_=xT[128 * k:128 * (k + 1), 512 * t:512 * (t + 1)],
                    )

            # ---- PSUM pools ----
            pa = ctx.enter_context(tc.tile_pool(name="pa", bufs=3, space="PSUM"))
            ps = ctx.enter_context(tc.tile_pool(name="ps", bufs=2, space="PSUM"))
            po = ctx.enter_context(tc.tile_pool(name="po", bufs=3, space="PSUM"))

            # working SBUF pools (PT: one tile per (b,h) -> no slot-reuse waits)
            pt_pool = ctx.enter_context(tc.tile_pool(name="ptp", bufs=BB * H))
            rp_pool = ctx.enter_context(tc.tile_pool(name="rpp", bufs=3))
            bc_pool = ctx.enter_context(tc.tile_pool(name="bcp", bufs=3))
            y_pool = ctx.enter_context(tc.tile_pool(name="yp", bufs=3))

            # ---- phase 1a: Q.T / K.T = W @ x.T, feature-major [C, tok] ----
            for which, w_sb, out_sb in (("q", wq_sb, qt_sb), ("k", wk_sb, kt_sb)):
                for t in range(NT4):
                    for co in range(NKC):
                        pqk = pa.tile([128, 512], F32, tag="pa", name=f"p{which}{co}_{t}")
                        for kc in range(NKC):
                            nc.tensor.matmul(
                                pqk,
                                w_sb[kc][:, 128 * co:128 * (co + 1)],
                                xt_sb[kc][t],
                                start=(kc == 0),
                                stop=(kc == NKC - 1),
                            )
                        if which == "q":
                            nc.scalar.copy(out_sb[co][t], pqk)
                        else:
                            nc.vector.tensor_copy(out_sb[co][t], pqk)

            # ---- phase 1b: V token-major [tok, C] ----
            for tb in range(2 * BB):
                pv = pa.tile([128, C], F32, tag="pa", name=f"pv{tb}")
                t4, off = tb // 4, (tb % 4) * 128
                for kc in range(NKC):
                    nc.tensor.matmul(
                        pv,
                        xt_sb[kc][t4][:, off:off + 128],
                        wv_sb[kc],
                        start=(kc == 0),
                        stop=(kc == NKC - 1),
                    )
                nc.scalar.copy(v_sb[tb], pv)

            # ---- phase 2: attention per (batch, head) ----
            for b in range(BB):
                t4b, qc = b // 2, (b % 2) * 256  # 512-tile index / col offset for this batch
                pzz = pa.tile([H, 256], F32, tag="pa", name=f"pzz{b}")
                po_tiles = []
                for hp in range(H // 2):  # head pairs
                    p_o = po.tile([64, 512], F32, tag="po", name=f"po{b}_{hp}")
                    po_tiles.append(p_o)
                    for hh in range(2):
                        h = 2 * hp + hh
                        r0 = 64 * hh
                        c0 = 256 * hh
                        qt = qt_sb[hp][t4b]
                        kt = kt_sb[hp][t4b]
                        # scores transposed: S.T[tk, tq] = K.T(d,tk).T @ Q.T(d,tq)
                        p_s = ps.tile([128, 384], F32, tag="ps", name=f"s{b}_{h}")
                        nc.tensor.matmul(
                            p_s[:, 0:256],
                            kt[r0:r0 + 64, qc:qc + 128],
                            qt[r0:r0 + 64, qc:qc + 256],
                            start=True, stop=True,
                        )
                        nc.tensor.matmul(
                            p_s[:, 256:384],
                            kt[r0:r0 + 64, qc + 128:qc + 256],
                            qt[r0:r0 + 64, qc + 128:qc + 256],
                            start=True, stop=True,
                        )
                        # P.T = exp(S.T / sqrt(C)); cols 0:256 = tk-blk0 x tq 0:256,
                        # cols 256:384 = tk-blk1 x tq 128:256
                        pt = pt_pool.tile([128, 384], BF16, tag="pt", name=f"pt{b}_{h}")
                        nc.scalar.activation(pt, p_s, mybir.ActivationFunctionType.Exp, scale=SCALE)
                        # causal mask on the two diagonal blocks (cols 0:128, 256:384):
                        # keep where tq_within_block >= tk (GPSIMD affine select)
                        sel = pt.rearrange("p (a i) -> p a i", i=128)[:, 0::2, :]
                        nc.gpsimd.affine_select(
                            out=sel, in_=sel,
                            pattern=[[0, 2], [1, 128]],
                            compare_op=mybir.AluOpType.is_ge,
                            fill=0.0, base=0, channel_multiplier=-1,
                        )
                        # O.T[d, tq] (+= over tk blocks)
                        nc.tensor.matmul(
                            p_o[0:64, c0:c0 + 256],
                            v_sb[2 * b][:, 64 * h:64 * (h + 1)],
                            pt[:, 0:256],
                            start=True, stop=False,
                        )
                        nc.tensor.matmul(
                            p_o[0:64, c0 + 128:c0 + 256],
                            v_sb[2 * b + 1][:, 64 * h:64 * (h + 1)],
                            pt[:, 256:384],
                            start=False, stop=True,
                        )
                        # Z-gather: row h of pzz accumulates softmax denominators
                        nc.tensor.matmul(
                            pzz[0:H, 0:256],
                            ez_sb[:, H * h:H * (h + 1)],
                            pt[:, 0:256],
                            start=(h == 0), stop=False,
                            skip_group_check=True,
                        )
                        nc.tensor.matmul(
                            pzz[0:H, 128:256],
                            ez_sb[:, H * h:H * (h + 1)],
                            pt[:, 256:384],
                            start=False, stop=(h == H - 1),
                            skip_group_check=True,
                        )
                # denominators -> reciprocals (one op for all 6 heads of this batch)
                rp = rp_pool.tile([H, 256], F32, tag="rp", name=f"rp{b}")
                nc.vector.reciprocal_approx_fast(rp, pzz)
                # broadcast all 6 reciprocal rows to 64 partitions via a DRAM
                # round-trip (DRAM APs allow the partition-broadcast read)
                nc.sync.dma_start(
                    out=zdram[b:b + 1, :].rearrange("o (h q) -> (o h) q", q=256),
                    in_=rp,
                )
                bc = bc_pool.tile([64, H * 256], F32, tag="bc", name=f"bc{b}")
                zsrc = zdram[b, :]
                bc_src = bass.AP(
                    tensor=zsrc.tensor, offset=zsrc.offset,
                    ap=[[0, 64]] + [list(p) for p in zsrc.ap],
                )
                nc.sync.dma_start(out=bc, in_=bc_src)
                # normalize each head's O.T into catT
                for h in range(H):
                    hp, hh = h // 2, h % 2
                    c0 = 256 * hh
                    nc.vector.tensor_mul(
                        cat_sb[hp][t4b][64 * hh:64 * (hh + 1), qc:qc + 256],
                        po_tiles[hp][0:64, c0:c0 + 256],
                        bc[:, 256 * h:256 * (h + 1)],
                    )

            # ---- phase 3: y.T = Wo @ catT + bo ----
            for t in range(NT4):
                for co in range(NKC):
                    pyk = pa.tile([128, 512], F32, tag="pa", name=f"py{co}_{t}")
                    for kc in range(NKC):
                        nc.tensor.matmul(
                            pyk,
                            wo_sb[kc][:, 128 * co:128 * (co + 1)],
                            cat_sb[kc][t],
                            start=(kc == 0),
                            stop=(kc == NKC - 1),
                        )
                    yt = y_pool.tile([128, 512], F32, tag="yt", name=f"yt{co}_{t}")
                    nc.vector.tensor_scalar_add(yt, pyk, wob_sb[co][:, 0:1])
                    nc.sync.dma_start(
                        out=yT[128 * co:128 * (co + 1), 512 * t:512 * (t + 1)],
                        in_=yt,
                    )

    nc.compile()
    return nc


def make_in_maps(x, Wk, Wq, Wv, Wo, bo):
    x = np.asarray(x, np.float32)
    wqT = np.ascontiguousarray(np.asarray(Wq, np.float32).T).astype(NPBF)
    wkT = np.ascontiguousarray(np.asarray(Wk, np.float32).T).astype(NPBF)
    wvT = np.ascontiguousarray(np.asarray(Wv, np.float32).T).astype(NPBF)
    woT = np.ascontiguousarray(np.asarray(Wo, np.float32).T).astype(NPBF)
    wobc = np.ascontiguousarray(np.asarray(bo, np.float32).reshape(C, 1))
    ez = np.zeros((128, H * H), NPBF)
    for h in range(H):
        ez[:, H * h + h] = 1.0
    # mask keeps tq >= tk in [tk, tq] layout -> upper triangular incl diagonal
    tl = np.triu(np.ones((128, 128), np.float32))
    tril = np.concatenate([tl, tl], axis=1).astype(NPBF)
    in_maps = []
    for i in range(NCORES):
        xi = x[BB * i:BB * (i + 1)].reshape(TOK, C)
        in_maps.append({
            "xT": np.ascontiguousarray(xi.T).astype(NPBF),
            "wqT": wqT, "wkT": wkT, "wvT": wvT, "woT": woT,
            "wobc": wobc, "ez": ez, "tril": tril,
        })
    return in_maps


_NC_CACHE = None


def kernel(x, Wk, Wq, Wv, Wo, bo):
    global _NC_CACHE
    if _NC_CACHE is None:
        _NC_CACHE = build_module()
    nc = _NC_CACHE
    in_maps = make_in_maps(x, Wk, Wq, Wv, Wo, bo)
    res = run_bass_kernel_spmd(nc, in_maps, core_ids=list(range(NCORES)))
    outs = []
    for i in range(NCORES):
        yt = np.asarray(res.results[i]["yT"])
        outs.append(yt.T.reshape(BB, T, C))
    return np.concatenate(outs, axis=0).astype(np.float32)


# revision 15
# speedup vs baseline: 3.9463x; 1.0790x over previous
"""Multi-head causal self-attention (B=64, T=256, C=384, H=6) on 8 NeuronCores.

Data-parallel over batch: each core processes 8 batches (2048 tokens).
All on-device tensors are laid out so no device-side transposes are needed:
  - xT, Q.T, K.T feature-major [C, tokens]
  - V token-major [tokens, C]
  - scores computed transposed (S.T[tk, tq]) so exp(S.T) feeds P.T@V directly
  - attention output lands feature-major (catT) for the output projection
Matmul operands are bf16 (fp32 matmul is two-pass on trn2); accumulation,
softmax denominators, normalization and the final output stay fp32.
Heads are processed in pairs: score matmuls row-pack (K=64 at partition 0/64),
O.T matmuls col-pack (M=64 at array columns 0/64) into one [128,256] PSUM
tile, so softmax normalization is one tensor_tensor per pair.
"""

import sys

import ml_dtypes
import numpy as np

for _p in ("/opt/trn_rl_repo", "/root/.axon_site/_ro/trn_rl_repo"):
    if _p not in sys.path:
        sys.path.insert(0, _p)

import concourse.bass as bass
import concourse.tile as tile
from concourse import bacc, mybir
from concourse.bass_utils import run_bass_kernel_spmd

B, T, C, H, D = 64, 256, 384, 6, 64
NCORES = 8
BB = B // NCORES  # batches per core = 8
TOK = BB * T      # tokens per core = 2048
SCALE = float(C) ** -0.5
F32 = mybir.dt.float32
BF16 = mybir.dt.bfloat16
NPBF = ml_dtypes.bfloat16

NT4 = TOK // 512  # 4 column-chunks of 512 tokens
NKC = C // 128    # 3 chunks of 128 over feature dim


def build_module():
    nc = bacc.Bacc("TRN2", target_bir_lowering=False, debug=False)

    xT = nc.dram_tensor("xT", [C, TOK], BF16, kind="ExternalInput").ap()
    wall = nc.dram_tensor("wall", [C, 4 * C], BF16, kind="ExternalInput").ap()
    wobc = nc.dram_tensor("wobc", [C, 1], F32, kind="ExternalInput").ap()
    ez = nc.dram_tensor("ez", [128, H * H], BF16, kind="ExternalInput").ap()
    yT = nc.dram_tensor("yT", [C, TOK], F32, kind="ExternalOutput").ap()
    # DRAM scratch for the per-batch reciprocal rows (enables broadcast DMA)
    zdram = nc.dram_tensor("zdram", [BB, H * 256], F32).ap()

    with tile.TileContext(nc) as tc:
        import contextlib

        ctx = contextlib.ExitStack()
        with ctx:
            consts = ctx.enter_context(tc.tile_pool(name="consts", bufs=1))

            # ---- persistent SBUF tiles ----
            def ptile(name, shape, dt=BF16):
                return consts.tile(shape, dt, tag=name, name=name)

            wall_sb = [ptile(f"wall{k}", [128, 4 * C]) for k in range(NKC)]
            wq_sb = [w[:, 0:C] for w in wall_sb]
            wk_sb = [w[:, C:2 * C] for w in wall_sb]
            wv_sb = [w[:, 2 * C:3 * C] for w in wall_sb]
            wo_sb = [w[:, 3 * C:4 * C] for w in wall_sb]
            wob_sb = [ptile(f"wob{k}", [128, 1], F32) for k in range(NKC)]
            ez_sb = ptile("ez", [128, H * H])
            xt_sb = [ptile(f"xt{k}", [128, TOK]) for k in range(NKC)]
            qt_sb = [[ptile(f"qt{k}_{t}", [128, 512]) for t in range(NT4)] for k in range(NKC)]
            kt_sb = [[ptile(f"kt{k}_{t}", [128, 512]) for t in range(NT4)] for k in range(NKC)]
            cat_sb = [[ptile(f"cat{k}_{t}", [128, 512]) for t in range(NT4)] for k in range(NKC)]
            v_sb = [ptile(f"v{t}", [128, C]) for t in range(2 * BB)]  # 16 token-blocks of 128

            # ---- input DMAs ----
            for k in range(NKC):
                nc.gpsimd.dma_start(out=wall_sb[k], in_=wall[128 * k:128 * (k + 1), :])
                nc.gpsimd.dma_start(out=wob_sb[k], in_=wobc[128 * k:128 * (k + 1), :])
            nc.gpsimd.dma_start(out=ez_sb, in_=ez)
            for k in range(NKC):
                nc.sync.dma_start(out=xt_sb[k], in_=xT[128 * k:128 * (k + 1), :])

            # ---- PSUM pools (8 banks: pa 2 + ps 2x2 + po 2) ----
            pa = ctx.enter_context(tc.tile_pool(name="pa", bufs=2, space="PSUM"))
            ps = ctx.enter_context(tc.tile_pool(name="ps", bufs=2, space="PSUM"))
            po = ctx.enter_context(tc.tile_pool(name="po", bufs=2, space="PSUM"))

            # working SBUF pools (PT: one tile per (b,pair) -> no slot-reuse waits)
            pt_pool = ctx.enter_context(tc.tile_pool(name="ptp", bufs=BB * H // 2))
            rp_pool = ctx.enter_context(tc.tile_pool(name="rpp", bufs=3))
            bc_pool = ctx.enter_context(tc.tile_pool(name="bcp", bufs=3))
            y_pool = ctx.enter_context(tc.tile_pool(name="yp", bufs=3))

            # ---- phase 1a: Q.T / K.T = W @ x.T, feature-major [C, tok] ----
            for t in range(NT4):
                for which, w_sb, out_sb in (("q", wq_sb, qt_sb), ("k", wk_sb, kt_sb)):
                    for co in range(NKC):
                        pqk = pa.tile([128, 512], F32, tag="pa", name=f"p{which}{co}_{t}")
                        for kc in range(NKC):
                            nc.tensor.matmul(
                                pqk,
                                w_sb[kc][:, 128 * co:128 * (co + 1)],
                                xt_sb[kc][:, 512 * t:512 * (t + 1)],
                                start=(kc == 0),
                                stop=(kc == NKC - 1),
                            )
                        if which == "q":
                            nc.scalar.copy(out_sb[co][t], pqk)
                        else:
                            nc.vector.tensor_copy(out_sb[co][t], pqk)

            # ---- phase 1b: V token-major [tok, C] ----
            for tb in range(2 * BB):
                pv = pa.tile([128, C], F32, tag="pa", name=f"pv{tb}")
                for kc in range(NKC):
                    nc.tensor.matmul(
                        pv,
                        xt_sb[kc][:, 128 * tb:128 * (tb + 1)],
                        wv_sb[kc],
                        start=(kc == 0),
                        stop=(kc == NKC - 1),
                    )
                nc.scalar.copy(v_sb[tb], pv)

            # ---- phase 2: attention, head pairs ----
            for b in range(BB):
                t4b, qc = b // 2, (b % 2) * 256  # 512-tile index / col offset for this batch
                pzz = pa.tile([H, 256], F32, tag="pa", name=f"pzz{b}")
                po_tiles = []
                for hp in range(H // 2):
                    # scores for the pair: h0 at cols 0:384 (bank 0), h1 at 512:896
                    # (bank 1); within a head: cols +0:256 = tk-blk0 x tq 0:256,
                    # cols +256:384 = tk-blk1 x tq 128:256
                    p_s = ps.tile([128, 1024], F32, tag="ps", name=f"s{b}_{hp}")
                    for hh in range(2):
                        h = 2 * hp + hh
                        r0, s0 = 64 * hh, 512 * hh
                        qt = qt_sb[hp][t4b]
                        kt = kt_sb[hp][t4b]
                        nc.tensor.matmul(
                            p_s[:, s0:s0 + 256],
                            kt[r0:r0 + 64, qc:qc + 128],
                            qt[r0:r0 + 64, qc:qc + 256],
                            start=True, stop=True,
                        )
                        nc.tensor.matmul(
                            p_s[:, s0 + 256:s0 + 384],
                            kt[r0:r0 + 64, qc + 128:qc + 256],
                            qt[r0:r0 + 64, qc + 128:qc + 256],
                            start=True, stop=True,
                        )
                    # P.T = exp(S.T / sqrt(C)) for both heads in one op
                    pt = pt_pool.tile([128, 768], BF16, tag="pt", name=f"pt{b}_{hp}")
                    nc.scalar.activation(
                        pt.rearrange("p (a q) -> p a q", q=384),
                        p_s.rearrange("p (a q) -> p a q", q=512)[:, :, 0:384],
                        mybir.ActivationFunctionType.Exp, scale=SCALE,
                    )
                    # causal mask on the diagonal blocks (ISA allows 2 free dims)
                    for hh in range(2):
                        sel = pt[:, 384 * hh:384 * (hh + 1)] \
                            .rearrange("p (c i) -> p c i", i=128)[:, 0::2, :]
                        nc.gpsimd.affine_select(
                            out=sel, in_=sel,
                            pattern=[[0, 2], [1, 128]],
                            compare_op=mybir.AluOpType.is_ge,
                            fill=0.0, base=0, channel_multiplier=-1,
                        )
                    # O.T for the pair: h0 -> psum rows 0:64 (array cols 0:64),
                    # h1 -> psum rows 64:128 (array cols 64:128)
                    p_o = po.tile([128, 256], F32, tag="po", name=f"po{b}_{hp}")
                    po_tiles.append(p_o)
                    for tkb in range(2):
                        for hh in range(2):
                            h = 2 * hp + hh
                            r0, s0 = 64 * hh, 512 * hh
                            if tkb == 0:
                                nc.tensor.matmul(
                                    p_o[r0:r0 + 64, 0:256],
                                    v_sb[2 * b][:, 64 * h:64 * (h + 1)],
                                    pt[:, 384 * hh:384 * hh + 256],
                                    start=True, stop=False,
                                    tile_position=(0, r0),
                                    skip_group_check=True,
                                )
                            else:
                                nc.tensor.matmul(
                                    p_o[r0:r0 + 64, 128:256],
                                    v_sb[2 * b + 1][:, 64 * h:64 * (h + 1)],
                                    pt[:, 384 * hh + 256:384 * hh + 384],
                                    start=False, stop=True,
                                    tile_position=(0, r0),
                                    skip_group_check=True,
                                )
                    # Z-gather: row h of pzz accumulates softmax denominators
                    for hh in range(2):
                        h = 2 * hp + hh
                        nc.tensor.matmul(
                            pzz[0:H, 0:256],
                            ez_sb[:, H * h:H * (h + 1)],
                            pt[:, 384 * hh:384 * hh + 256],
                            start=(h == 0), stop=False,
                            skip_group_check=True,
                        )
                        nc.tensor.matmul(
                            pzz[0:H, 128:256],
                            ez_sb[:, H * h:H * (h + 1)],
                            pt[:, 384 * hh + 256:384 * hh + 384],
                            start=False, stop=(h == H - 1),
                            skip_group_check=True,
                        )
                # denominators -> reciprocals (one op for all 6 heads of this batch)
                rp = rp_pool.tile([H, 256], F32, tag="rp", name=f"rp{b}")
                nc.vector.reciprocal_approx_fast(rp, pzz)
                # broadcast reciprocal rows to a [128, 3*256] tile via DRAM:
                # rows 0:64 = even heads, rows 64:128 = odd heads, pair-major cols
                nc.sync.dma_start(
                    out=zdram[b:b + 1, :].rearrange("o (h q) -> (o h) q", q=256),
                    in_=rp,
                )
                bc = bc_pool.tile([128, (H // 2) * 256], F32, tag="bc", name=f"bc{b}")
                zsrc = zdram[b, :]
                for hh in range(2):
                    bc_src = bass.AP(
                        tensor=zsrc.tensor, offset=zsrc.offset + 256 * hh,
                        ap=[[0, 64], [512, H // 2], [1, 256]],
                    )
                    nc.sync.dma_start(out=bc[64 * hh:64 * (hh + 1), :], in_=bc_src)
                # normalize each pair's O.T into catT (one op per pair)
                for hp in range(H // 2):
                    nc.vector.tensor_mul(
                        cat_sb[hp][t4b][:, qc:qc + 256],
                        po_tiles[hp],
                        bc[:, 256 * hp:256 * (hp + 1)],
                    )

            # ---- phase 3: y.T = Wo @ catT + bo ----
            for t in range(NT4):
                for co in range(NKC):
                    pyk = pa.tile([128, 512], F32, tag="pa", name=f"py{co}_{t}")
                    for kc in range(NKC):
                        nc.tensor.matmul(
                            pyk,
                            wo_sb[kc][:, 128 * co:128 * (co + 1)],
                            cat_sb[kc][t],
                            start=(kc == 0),
                            stop=(kc == NKC - 1),
                        )
                    yt = y_pool.tile([128, 512], F32, tag="yt", name=f"yt{co}_{t}")
                    nc.vector.tensor_scalar_add(yt, pyk, wob_sb[co][:, 0:1])
                    nc.sync.dma_start(
                        out=yT[128 * co:128 * (co + 1), 512 * t:512 * (t + 1)],
                        in_=yt,
                    )

    nc.compile()
    return nc


def make_in_maps(x, Wk, Wq, Wv, Wo, bo):
    x = np.asarray(x, np.float32)
    wall = np.concatenate(
        [np.asarray(w, np.float32).T for w in (Wq, Wk, Wv, Wo)], axis=1
    ).astype(NPBF)
    wobc = np.ascontiguousarray(np.asarray(bo, np.float32).reshape(C, 1))
    ez = np.zeros((128, H * H), NPBF)
    for h in range(H):
        ez[:, H * h + h] = 1.0
    in_maps = []
    for i in range(NCORES):
        xi = x[BB * i:BB * (i + 1)].reshape(TOK, C)
        in_maps.append({
            "xT": np.ascontiguousarray(xi.T).astype(NPBF),
            "wall": wall, "wobc": wobc, "ez": ez,
        })
    return in_maps


_NC_CACHE = None


def kernel(x, Wk, Wq, Wv, Wo, bo):
    global _NC_CACHE
    if _NC_CACHE is None:
        _NC_CACHE = build_module()
    nc = _NC_CACHE
    in_maps = make_in_maps(x, Wk, Wq, Wv, Wo, bo)
    res = run_bass_kernel_spmd(nc, in_maps, core_ids=list(range(NCORES)))
    outs = []
    for i in range(NCORES):
        yt = np.asarray(res.results[i]["yT"])
        outs.append(yt.T.reshape(BB, T, C))
    return np.concatenate(outs, axis=0).astype(np.float32)
